# revision 1
# baseline (speedup 1.0000x reference)
"""JTNN graph-encoder message passing on 8 Trainium2 NeuronCores, v2.

Key structure vs v1:
- Message table in bf16, rows padded to 512 cols (1024B) in two
  double-buffered Shared DRAM tables (T_A / T_B) so each iteration's
  AllGather overlaps the next iteration's compute (no WAR stall).
- Bulk gathers via gpsimd.dma_gather (int16 indices): two instructions
  per 128-bond tile (lo/hi table views to fit the int16 index range)
  instead of one indirect DMA per neighbor slot.
- Pad slots point at 64-row zero stripes at both ends of the table.
- AllGather runs in 4 chunks per iteration, bf16, Shared output.
- Tree-message contribution to the bond update is folded into binput
  once; atom readout gathers tree+graph rows from the final table.
"""
import sys
sys.path.insert(0, "/opt/trn_rl_repo")
import os
import numpy as np
import ml_dtypes

H = 450
HP = 512
DEPTH = 6
NT = 16000
NB = 40000
NBP = 40960
PB = NBP // 8        # 5120 bonds per core
TB = PB // 128       # 40 bond tiles per core
NA = 20000
PA = 2560
TA = PA // 128       # 20 atom tiles per core
AF = 35
BF = 40
MAXNB = 15
ZS = 64              # zero-stripe rows at each end of the table
TREE0 = ZS           # tree rows [64, 16064)
BOND0 = ZS + NT      # bond rows [16064, 57024)
NTOT = ZS + NT + NBP + ZS   # 57088
VA = 32768           # view A = rows [0, 32768)
VB0 = NTOT - 32768   # 24320; view B = rows [24320, 57088)
CHUNKS = int(os.environ.get("K_CHUNKS", "4"))
TPC = TB // CHUNKS   # tiles per AllGather chunk
CROWS = PB // CHUNKS  # 1280 per-core rows per chunk
GROWS = CROWS * 8    # 10240 global rows per chunk
N_CORES = 8

_CACHE = {}


def _build_program(KLG, KHG, KTR, KLA, KHA, KTA):
    import os
    STAGE = int(os.environ.get("K_STAGE", "9"))
    N_ITERS = int(os.environ.get("K_ITERS", DEPTH - 1))
    import concourse.bacc as bacc
    import concourse.bass as bass
    import concourse.mybir as mybir
    import concourse.tile as tile
    from concourse.masks import make_identity
    from concourse.library_config import mlp

    f32 = mybir.dt.float32
    bf16 = mybir.dt.bfloat16
    i16 = mybir.dt.int16
    Relu = mybir.ActivationFunctionType.Relu
    nc = bacc.Bacc(target_bir_lowering=False, num_devices=N_CORES)

    tree_sh = nc.dram_tensor("tree_sh", [NT // 8, HP], bf16, kind="ExternalInput")
    fbondsT = nc.dram_tensor("fbondsT", [BF, PB], f32, kind="ExternalInput")
    fatomsT = nc.dram_tensor("fatomsT", [AF, PA], bf16, kind="ExternalInput")
    W_iT = nc.dram_tensor("W_iT", [BF, H], f32, kind="ExternalInput")
    W_hT = nc.dram_tensor("W_hT", [HP, H], bf16, kind="ExternalInput")
    W_o1T = nc.dram_tensor("W_o1T", [AF, H], bf16, kind="ExternalInput")
    W_o2T = nc.dram_tensor("W_o2T", [HP, H], bf16, kind="ExternalInput")
    b_oT = nc.dram_tensor("b_oT", [H, 1], f32, kind="ExternalInput")
    idxm = nc.dram_tensor("idxm", [128, TB, 256], i16, kind="ExternalInput")
    idxt = nc.dram_tensor("idxt", [128, TB, 128], i16, kind="ExternalInput")
    idxa = nc.dram_tensor("idxa", [128, TA, 256], i16, kind="ExternalInput")
    idxat = nc.dram_tensor("idxat", [128, TA, 128], i16, kind="ExternalInput")
    out_ms = nc.dram_tensor("out_ms", [4, 128, 128], f32, kind="ExternalOutput")

    RG = [list(range(N_CORES))]
    KMAX = 1
    for t in range(TB):
        KMAX = max(KMAX, KLG[t] + KHG[t], KTR[t])
    for t in range(TA):
        KMAX = max(KMAX, KLA[t] + KHA[t], KTA[t])

    with tile.TileContext(nc) as tc:
        with (
            tc.tile_pool(name="const", bufs=1) as cp,
            tc.tile_pool(name="sbuf", bufs=int(os.environ.get("K_SPBUFS", "3"))) as sp,
            tc.tile_pool(name="lhs", bufs=2) as lp,
            tc.tile_pool(name="gt", bufs=int(os.environ.get("K_GTBUFS", "3"))) as gtp,
            tc.tile_pool(name="neit", bufs=8) as ntp,
            tc.tile_pool(name="psum", bufs=4, space="PSUM") as pp,
            tc.tile_pool(name="apsum", bufs=2, space="PSUM") as app,
            tc.tile_pool(name="ypsum", bufs=int(os.environ.get("K_YPBUFS", "2")), space="PSUM") as ypp,
            tc.tile_pool(name="dram", bufs=1, space="DRAM") as dp,
            nc.allow_low_precision(reason="bf16 message table"),
        ):
            nc.gpsimd.load_library(mlp)
            T_A = nc.dram_tensor("T_A", [NTOT, HP], bf16,
                                 kind="Internal", addr_space="Shared")
            T_B = nc.dram_tensor("T_B", [NTOT, HP], bf16,
                                 kind="Internal", addr_space="Shared")
            # per-chunk y buffers, double-buffered by iteration parity, so a
            # chunk's AllGather only depends on that chunk's scatters
            ycb = []
            for par in range(2):
                row = []
                for g in range(CHUNKS):
                    yc = dp.tile([CROWS, HP], bf16, name=f"ycb{par}_{g}")
                    row.append(yc)
                ycb.append(row)
            tree_b = dp.tile([NT // 8, HP], bf16)

            identf = cp.tile([128, 128], f32)
            make_identity(nc, identf[:])
            ident = cp.tile([128, 128], bf16)
            nc.vector.tensor_copy(ident[:], identf[:])
            w_it = cp.tile([BF, H], f32)
            nc.sync.dma_start(w_it[:], W_iT[:])
            w_ht = cp.tile([128, 4, H], bf16)
            nc.sync.dma_start(w_ht[:], W_hT[:].rearrange("(c k) h -> k c h", k=128))
            w_o1t = cp.tile([AF, H], bf16)
            nc.sync.dma_start(w_o1t[:], W_o1T[:])
            w_o2t = cp.tile([128, 4, H], bf16)
            nc.sync.dma_start(w_o2t[:], W_o2T[:].rearrange("(c k) h -> k c h", k=128))
            b_ot = cp.tile([128, 4], f32)
            nc.vector.memset(b_ot[:], 0.0)
            nc.sync.dma_start(b_ot[:, 0:3],
                              b_oT[0:384, :].rearrange("(c k) o -> k (c o)", k=128))
            nc.sync.dma_start(b_ot[0:66, 3:4], b_oT[384:450, :])
            igm = cp.tile([128, TB, 256], i16)
            nc.sync.dma_start(igm[:], idxm[:])
            igt = cp.tile([128, TB, 128], i16)
            nc.sync.dma_start(igt[:], idxt[:])
            iga = cp.tile([128, TA, 256], i16)
            nc.sync.dma_start(iga[:], idxa[:])
            igat = cp.tile([128, TA, 128], i16)
            nc.sync.dma_start(igat[:], idxat[:])
            atree = dp.tile([PA, HP], bf16)
            fat = cp.tile([AF, PA], bf16)
            nc.sync.dma_start(fat[:], fatomsT[:])
            binput = cp.tile([128, TB, H], bf16)

            # zero stripes at both ends of both tables
            zr = sp.tile([64, HP], bf16, tag="zr")
            nc.vector.memset(zr[:], 0.0)
            for T in (T_A, T_B):
                nc.sync.dma_start(T[0:ZS, :], zr[:])
                nc.sync.dma_start(T[NTOT - ZS:NTOT, :], zr[:])
            # tree shard -> bounce -> AllGather into T_A only. T_B's tree
            # region is never read: iteration gathers index only bond rows
            # and stripes, and the atom tree contribution is prefolded from
            # T_A — so no second tree AllGather.
            nc.gpsimd.dma_start(tree_b[:], tree_sh[:])
            nc.gpsimd.collective_compute(
                "AllGather", mybir.AluOpType.bypass, replica_groups=RG,
                ins=[tree_b[:]], outs=[T_A[TREE0:BOND0, :]])

            def gather_reduce(idx_tile, t, KL, KH, src, lo_end=VA):
                """One [128, HP] bf16 tile = sum of KL lo + KH hi rows."""
                gt = gtp.tile([128, KMAX, HP], bf16, tag="gt")
                if KL > 0:
                    nc.gpsimd.dma_gather(
                        gt[:, 0:KL, :], src[0:lo_end, :],
                        idx_tile[:, t, 0:KL * 8],
                        KL * 128, KL * 128, HP, single_packet=False)
                if KH > 0:
                    nc.gpsimd.dma_gather(
                        gt[:, KL:KL + KH, :], src[VB0:NTOT, :],
                        idx_tile[:, t, 128:128 + KH * 8],
                        KH * 128, KH * 128, HP, single_packet=False)
                m = KL + KH
                while m > 1:
                    h = m // 2
                    nc.vector.tensor_add(
                        gt[:, :h, :], gt[:, :h, :], gt[:, m - h:m, :])
                    m = m - h
                return gt

            def transpose4(acc, tag):
                nts = []
                for c in range(4):
                    tp = pp.tile([128, 128], bf16, tag="tp")
                    nc.tensor.transpose(tp[:], acc[:, c * 128:(c + 1) * 128],
                                        ident[:])
                    nt_ = ntp.tile([128, 128], bf16, tag=tag)
                    nc.vector.tensor_copy(nt_[:], tp[:])
                    nts.append(nt_)
                return nts

            def transform(acc, extra=None):
                """psum_y [128, H] f32 = acc[128, HP]bf16 @ W_h.T (+ extra).

                `extra` [128, H] bf16 is accumulated into the same PSUM via
                an identity matmul, saving a DVE add on the critical path.
                """
                psy = ypp.tile([128, H], f32, tag="yps")
                nts = transpose4(acc, "ntb")
                last = extra is None
                for c in range(4):
                    nc.tensor.matmul(psy[:], lhsT=nts[c][:], rhs=w_ht[:, c, :],
                                     start=(c == 0), stop=(c == 3 and last))
                if extra is not None:
                    nc.tensor.matmul(psy[:], lhsT=ident[:], rhs=extra,
                                     start=False, stop=True)
                return psy, nts

            def scatter_y(y, t, par):
                r0 = (t % TPC) * 128
                nc.sync.dma_start(ycb[par][t // TPC][r0:r0 + 128, :], y[:])

            def ag_chunk(t, par, dst):
                g = t // TPC
                nc.gpsimd.collective_compute(
                    "AllGather", mybir.AluOpType.bypass, replica_groups=RG,
                    ins=[ycb[par][g][:]],
                    outs=[dst[BOND0 + g * GROWS:BOND0 + (g + 1) * GROWS, :]])

            # binput = fbonds @ W_i.T ; y0 = relu(binput); chunked AG0
            for t in range(TB if STAGE >= 1 else 0):
                fb = lp.tile([BF, 128], f32, tag="fb")
                nc.sync.dma_start(fb[:], fbondsT[:, t * 128:(t + 1) * 128])
                ps = ypp.tile([128, H], f32, tag="yps")
                nc.tensor.matmul(ps[:], lhsT=fb[:], rhs=w_it[:],
                                 start=True, stop=True)
                nc.vector.tensor_copy(binput[:, t, :], ps[:])
                y0 = sp.tile([128, HP], bf16, tag="y")
                nc.vector.memset(y0[:, H:HP], 0.0)
                nc.scalar.activation(y0[:, 0:H], ps[:], Relu)
                scatter_y(y0, t, 0)
                if t % TPC == TPC - 1:
                    ag_chunk(t, 0, T_A)

            # fold constant tree contribution into binput. The narrow view
            # [0:BOND0) covers all tree indices and avoids a false
            # dependency on AG0's bond-region writes, so the fold overlaps
            # the y0/AG0 phase.
            for t in range(TB if STAGE >= 2 else 0):
                if KTR[t] == 0:
                    continue
                gt = gather_reduce(igt, t, KTR[t], 0, T_A, lo_end=BOND0)
                psy, _ = transform(gt[:, 0, :])
                nc.vector.tensor_add(binput[:, t, :], binput[:, t, :], psy[:])

            # atom tree-neighbor contribution: static, gathered early from
            # T_A's tree region only (no bond-region dep), overlapping iters
            for t in range(TA if STAGE >= 6 else 0):
                if KTA[t] > 0:
                    gt = gtp.tile([128, KMAX, HP], bf16, tag="gt")
                    nc.gpsimd.dma_gather(
                        gt[:, 0:KTA[t], :], T_A[0:BOND0, :],
                        igat[:, t, 0:KTA[t] * 8],
                        KTA[t] * 128, KTA[t] * 128, HP, single_packet=False)
                    m = KTA[t]
                    while m > 1:
                        h = m // 2
                        nc.vector.tensor_add(gt[:, :h, :], gt[:, :h, :],
                                             gt[:, m - h:m, :])
                        m = m - h
                    nc.sync.dma_start(atree[t * 128:(t + 1) * 128, :],
                                      gt[:, 0, :])
                else:
                    z0 = sp.tile([128, HP], bf16, tag="y")
                    nc.vector.memset(z0[:], 0.0)
                    nc.sync.dma_start(atree[t * 128:(t + 1) * 128, :], z0[:])

            # message update iterations
            n_iters = N_ITERS if STAGE >= 3 else 0
            for i in range(1, n_iters + 1):
                src = T_A if i % 2 == 1 else T_B
                dst = T_B if i % 2 == 1 else T_A
                par = i % 2
                for t in range(TB):
                    y = sp.tile([128, HP], bf16, tag="y")
                    nc.vector.memset(y[:, H:HP], 0.0)
                    if KLG[t] + KHG[t] > 0:
                        gt = gather_reduce(igm, t, KLG[t], KHG[t], src)
                        psy, _ = transform(gt[:, 0, :], extra=binput[:, t, :])
                        nc.scalar.activation(y[:, 0:H], psy[:], Relu)
                    else:
                        # real bonds with tree-only neighbors: y = relu(binput)
                        nc.scalar.activation(y[:, 0:H], binput[:, t, :], Relu)
                    scatter_y(y, t, par)
                    if t % TPC == TPC - 1:
                        ag_chunk(t, par, dst)

            # atom readout from the table holding the final messages
            Tfin = T_B if n_iters % 2 == 1 else T_A
            ahT = cp.tile([128, 4, PA], bf16)
            nc.vector.memset(ahT[:], 0.0)
            for t in range(TA if STAGE >= 6 else 0):
                tat = lp.tile([128, HP], bf16, tag="tat")
                nc.sync.dma_start(tat[:], atree[t * 128:(t + 1) * 128, :])
                if KLA[t] + KHA[t] > 0:
                    gt = gather_reduce(iga, t, KLA[t], KHA[t], Tfin)
                    accv = gt[:, 0, :]
                    nc.vector.tensor_add(accv, accv, tat[:])
                else:
                    accv = tat[:]
                nts = transpose4(accv, "nta")
                for j in range(4):
                    jw = min(128, H - j * 128)
                    ps = app.tile([128, 128], f32, tag="aps")
                    nc.tensor.matmul(ps[:jw, :],
                                     lhsT=w_o1t[:, j * 128:j * 128 + jw],
                                     rhs=fat[:, t * 128:(t + 1) * 128],
                                     start=True, stop=False)
                    for c in range(4):
                        nc.tensor.matmul(ps[:jw, :],
                                         lhsT=w_o2t[:, c, j * 128:j * 128 + jw],
                                         rhs=nts[c][:], start=False, stop=(c == 3))
                    nc.scalar.activation(ahT[:jw, j, t * 128:(t + 1) * 128],
                                         ps[:jw, :], Relu,
                                         bias=b_ot[:jw, j:j + 1])
            # segment sums over 20-atom molecules -> [128, 128] per j-chunk
            for j in range(4):
                red = sp.tile([128, 128], f32, tag="red")
                nc.vector.tensor_reduce(
                    red[:], ahT[:, j, :].rearrange("p (m a) -> p m a", a=20),
                    axis=mybir.AxisListType.X, op=mybir.AluOpType.add)
                nc.sync.dma_start(out_ms[j], red[:])

    nc.compile()
    return nc


def _wrap(flat):
    """flat[j] = index for slot (p=j%128, c=j//128) -> dma_gather layout."""
    w = flat.reshape(-1, 16).T                    # [16, len/16]
    return np.tile(w, (8, 1)).astype(np.int16)    # [128, len/16]


def _prep(inputs):
    fatoms = np.asarray(inputs["fatoms"], np.float32)
    fbonds = np.asarray(inputs["fbonds"], np.float32)
    agraph = np.asarray(inputs["agraph"], np.int64)
    bgraph = np.asarray(inputs["bgraph"], np.int64)
    mol_ids = np.asarray(inputs["mol_ids"], np.int32)
    n_mols = int(inputs["n_mols"])
    tree = np.asarray(inputs["tree_message"], np.float32)
    W_i = np.asarray(inputs["W_i"], np.float32)
    W_h = np.asarray(inputs["W_h"], np.float32)
    W_o = np.asarray(inputs["W_o"], np.float32)
    b_o = np.asarray(inputs["b_o"], np.float32)
    bf = ml_dtypes.bfloat16

    n_graph = (bgraph >= NT).sum(1)
    order = np.argsort(-n_graph, kind="stable")
    order_p = np.concatenate([order, np.arange(NB, NBP)])
    # position pos -> (core, ltile, slot) -> global table row
    pos = np.arange(NBP)
    gt_ = pos // 128
    core_of = gt_ % 8
    lt = gt_ // 8
    slot = pos % 128
    grow_pos = (BOND0 + (lt // TPC) * GROWS + core_of * CROWS
                + (lt % TPC) * 128 + slot)
    row_of_bond = np.empty(NBP, np.int64)
    row_of_bond[order_p] = grow_pos

    def remap(idx):
        out = np.where(idx < NT, TREE0 + idx, 0)
        g = idx >= NT
        out[g] = row_of_bond[idx[g] - NT]
        return out

    isg = bgraph >= NT

    def split_counts(bg_rows, ag_rows):
        # iteration gathers: key 0 = lo graph, 1 = hi graph, 2 = tree/invalid
        key = np.where(~isg, 2, np.where(bg_rows < VA, 0, 1))
        srt = np.argsort(key, axis=1, kind="stable")
        vals_g = np.take_along_axis(bg_rows, srt, axis=1)
        Lg = (key == 0).sum(1)
        Hg = (key == 1).sum(1)
        vals_g = np.concatenate([vals_g, np.zeros((NBP - NB, MAXNB), np.int64)])
        Lg = np.concatenate([Lg, np.zeros(NBP - NB, np.int64)])
        Hg = np.concatenate([Hg, np.zeros(NBP - NB, np.int64)])
        # tree-fold gathers: tree neighbors only
        keyt = np.where(isg, 1, 0)
        srtt = np.argsort(keyt, axis=1, kind="stable")
        vals_t = np.take_along_axis(bg_rows, srtt, axis=1)
        Tg = (keyt == 0).sum(1)
        vals_t = np.concatenate([vals_t, np.zeros((NBP - NB, MAXNB), np.int64)])
        Tg = np.concatenate([Tg, np.zeros(NBP - NB, np.int64)])
        # atom gathers: tree part vs graph lo/hi
        keya = np.where(ag_rows < BOND0, 0, np.where(ag_rows < VA, 1, 2))
        srta = np.argsort(keya, axis=1, kind="stable")
        vals_a = np.take_along_axis(ag_rows, srta, axis=1)
        Ta_ = (keya == 0).sum(1)
        La = (keya == 1).sum(1)
        Ha = (keya == 2).sum(1)
        return vals_g, Lg, Hg, vals_t, Tg, vals_a, Ta_, La, Ha

    # Compute order == table order (y tiles written with plain DMAs, no
    # indirect scatter). Grouping keys (L, H) depend on the assignment
    # itself, so iterate sort -> assign -> recount twice: the second sort
    # uses near-exact keys, and indices/K-maxes are always recomputed
    # exactly from the final assignment.
    _, Lk, Hk, _, _, _, _, _, _ = split_counts(remap(bgraph), remap(agraph))
    t_arr = np.repeat(np.arange(TB), 128)
    p_arr = np.tile(np.arange(128), TB)
    base_pools = []
    for c in range(8):
        pools_c = []
        for g in range(CHUNKS):
            lts = np.arange(g * TPC, (g + 1) * TPC)
            pool_pos = ((np.repeat(lts, 128) * 8 + c) * 128
                        + np.tile(np.arange(128), TPC))
            pools_c.append(order_p[pool_pos])
        base_pools.append(pools_c)
    for _pass in range(2):
        comp_bonds = []       # [8][PB] bond ids in compute order
        for c in range(8):
            rows_c = []
            for g in range(CHUNKS):
                pool = base_pools[c][g]
                o = np.lexsort((-Hk[pool], -Lk[pool]))
                rows_c.append(pool[o])
            comp_bonds.append(np.concatenate(rows_c))
        for c in range(8):
            rows_final = (BOND0 + (t_arr // TPC) * GROWS + c * CROWS
                          + (t_arr % TPC) * 128 + p_arr)
            row_of_bond[comp_bonds[c]] = rows_final
        bg_rows = remap(bgraph)
        ag_rows = remap(agraph)
        vals_g, Lg, Hg, vals_t, Tg, vals_a, Ta_, La, Ha = \
            split_counts(bg_rows, ag_rows)
        Lk, Hk = Lg, Hg

    # per-tile max counts, uniform across cores (shared SPMD program)
    KLG = [0] * TB
    KHG = [0] * TB
    KTR = [0] * TB
    for t in range(TB):
        sel = np.concatenate([comp_bonds[c][t * 128:(t + 1) * 128]
                              for c in range(8)])
        KLG[t] = int(Lg[sel].max())
        KHG[t] = int(Hg[sel].max())
        KTR[t] = int(Tg[sel].max())
    KLA = [0] * TA
    KHA = [0] * TA
    KTA = [0] * TA
    APC = NA // 8  # 2500 real atoms per core
    for t in range(TA):
        lo = t * 128
        hi = min((t + 1) * 128, APC)
        if lo >= APC:
            continue
        sel = (np.arange(8)[:, None] * APC
               + np.arange(lo, hi)[None, :]).ravel()
        KLA[t] = int(La[sel].max())
        KHA[t] = int(Ha[sel].max())
        KTA[t] = int(Ta_[sel].max())

    pad_lo = (np.arange(128)[None, :] + 7 * np.arange(MAXNB)[:, None]) % ZS
    pad_hi = (NTOT - ZS - VB0) + pad_lo            # stripe1 in view B

    def block(vals128, cnt128, K, hi_view):
        """vals128 [128, 15] sorted rows; build flat [K*128] int16."""
        if K == 0:
            return None
        cols = vals128[:, :K].T.copy()             # [K, 128]
        if hi_view:
            cols = cols - VB0
        mask = np.arange(K)[:, None] < cnt128[None, :]
        pads = pad_hi[:K] if hi_view else pad_lo[:K]
        out = np.where(mask, cols, pads)
        assert out.min() >= 0 and out.max() < 32768
        return out.ravel().astype(np.int16)

    in_maps = []
    W_hTp = np.zeros((HP, H), np.float32)
    W_hTp[:H] = W_h.T
    W_o2Tp = np.zeros((HP, H), np.float32)
    W_o2Tp[:H] = W_o[:, AF:].T
    fb_p = np.concatenate([fbonds, np.zeros((NBP - NB, BF), np.float32)])
    counts = np.bincount(mol_ids, minlength=n_mols).astype(np.float32)

    for c in range(8):
        bonds_c = comp_bonds[c]                   # [PB] compute order
        idxm = np.zeros((128, TB, 256), np.int16)
        idxt = np.zeros((128, TB, 128), np.int16)
        for t in range(TB):
            b128 = bonds_c[t * 128:(t + 1) * 128]
            bl = block(vals_g[b128], Lg[b128], KLG[t], False)
            if bl is not None:
                # hi slots of vals_g start at Lg per bond; realign: lo block
                idxm[:, t, 0:KLG[t] * 8] = _wrap(bl)
            if KHG[t] > 0:
                colsv = np.zeros((128, KHG[t]), np.int64)
                for p in range(128):
                    b = b128[p]
                    h = int(Hg[b])
                    colsv[p, :h] = vals_g[b, Lg[b]:Lg[b] + h] - VB0
                    colsv[p, h:] = pad_hi[:KHG[t] - h, p]
                flat = colsv.T.ravel()
                assert flat.min() >= 0 and flat.max() < 32768
                idxm[:, t, 128:128 + KHG[t] * 8] = _wrap(flat.astype(np.int16))
            bt = block(vals_t[b128], Tg[b128], KTR[t], False)
            if bt is not None:
                idxt[:, t, 0:KTR[t] * 8] = _wrap(bt)

        a0 = c * APC
        idxa = np.zeros((128, TA, 256), np.int16)
        idxat = np.zeros((128, TA, 128), np.int16)
        for t in range(TA):
            lo = t * 128
            n_real = max(0, min(128, APC - lo))
            va = np.zeros((128, MAXNB), np.int64)
            ta_ = np.zeros(128, np.int64)
            ca_ = np.zeros(128, np.int64)
            ha_ = np.zeros(128, np.int64)
            if n_real > 0:
                aa = a0 + lo + np.arange(n_real)
                va[:n_real] = vals_a[aa]
                ta_[:n_real] = Ta_[aa]
                ca_[:n_real] = La[aa]
                ha_[:n_real] = Ha[aa]
            if KTA[t] > 0:
                bl = block(va, ta_, KTA[t], False)
                idxat[:, t, 0:KTA[t] * 8] = _wrap(bl)
            if KLA[t] > 0:
                colsv = np.zeros((128, KLA[t]), np.int64)
                for p in range(128):
                    l_ = int(ca_[p])
                    colsv[p, :l_] = va[p, ta_[p]:ta_[p] + l_]
                    colsv[p, l_:] = pad_lo[:KLA[t] - l_, p]
                flat = colsv.T.ravel()
                assert flat.min() >= 0 and flat.max() < 32768
                idxa[:, t, 0:KLA[t] * 8] = _wrap(flat.astype(np.int16))
            if KHA[t] > 0:
                colsv = np.zeros((128, KHA[t]), np.int64)
                for p in range(128):
                    h = int(ha_[p])
                    o = int(ta_[p] + ca_[p])
                    colsv[p, :h] = va[p, o:o + h] - VB0
                    colsv[p, h:] = pad_hi[:KHA[t] - h, p]
                flat = colsv.T.ravel()
                assert flat.min() >= 0 and flat.max() < 32768
                idxa[:, t, 128:128 + KHA[t] * 8] = _wrap(flat.astype(np.int16))

        tree_c = np.zeros((NT // 8, HP), np.float32)
        tree_c[:, :H] = tree[c * (NT // 8):(c + 1) * (NT // 8)]
        fat_c = np.zeros((AF, PA), np.float32)
        fat_c[:, :APC] = fatoms[a0:a0 + APC].T
        in_maps.append({
            "tree_sh": tree_c.astype(bf),
            "fbondsT": np.ascontiguousarray(fb_p[bonds_c].T),
            "fatomsT": fat_c.astype(bf),
            "W_iT": np.ascontiguousarray(W_i.T),
            "W_hT": W_hTp.astype(bf),
            "W_o1T": np.ascontiguousarray(W_o[:, :AF].T).astype(bf),
            "W_o2T": W_o2Tp.astype(bf),
            "b_oT": np.ascontiguousarray(b_o[:, None]),
            "idxm": idxm,
            "idxt": idxt,
            "idxa": idxa,
            "idxat": idxat,
        })
    return in_maps, (tuple(KLG), tuple(KHG), tuple(KTR),
                     tuple(KLA), tuple(KHA), tuple(KTA)), counts


def get_program_and_maps(inputs):
    in_maps, K, counts = _prep(inputs)
    if K not in _CACHE:
        _CACHE[K] = _build_program(*[list(k) for k in K])
    return _CACHE[K], in_maps, counts


def postprocess(results, counts):
    outs = []
    for c in range(N_CORES):
        ms = np.asarray(results[c]["out_ms"])    # [4, 128(j), 128(mol)]
        msf = ms.reshape(HP, 128)[:H, :125]
        outs.append(msf.T)
    sums = np.concatenate(outs, axis=0)
    return (sums / counts[:, None]).astype(np.float32)


def kernel(**inputs) -> np.ndarray:
    from concourse.bass_utils import run_bass_kernel_spmd

    nc, in_maps, counts = get_program_and_maps(inputs)
    res = run_bass_kernel_spmd(nc, in_maps, core_ids=list(range(N_CORES)))
    return postprocess(res.results, counts)

